# revision 17
# baseline (speedup 1.0000x reference)
"""JiT/DiT transformer block (adaLN + attention + SwiGLU) on 8 TRN2 NeuronCores.

Data-parallel over batch: core i computes batch element i end-to-end; no
collectives. Activations kept "transposed" on device ([channel, seq]) so
per-channel modulation/bias are per-partition scalars; attention scores are
produced directly in [k, q] layout (softmax denominator via a ones-row
appended to V inside the AV matmul).

All big linear matmuls (qkv, v, proj, w12, w3, ada, AV) run fp8e4 with
MatmulPerfMode.DoubleRow (2 contraction k-tiles per instruction, 2x rate),
fp32 PSUM accumulation. Weights are host-prescaled by 64, activations by a
power-of-2 per tensor; the products are unscaled on the PSUM copy-out (the
combined factor folds into the existing scale/bias of that op). Scores stay
bf16. The residual stream stays fp32.

Scale conventions (host ``*`` prescale / device unscale):
  weights *64 | hT,h2T *8 | v_sb *16 (bias row = 16) | ohat *64 | gg *8
  qkv psum = 512*qk -> raw = ps/512 + b
  exp bias +ln2 (cancels in softmax ratio, keeps pT in fp8 normal range)
"""

import sys

sys.path.insert(0, "/opt/trn_rl_repo")

import math

import numpy as np
import ml_dtypes

import concourse.bacc as bacc
import concourse.bass as bass
import concourse.mybir as mybir
from concourse.tile import TileContext
from concourse.bass_utils import run_bass_kernel_spmd

F32 = mybir.dt.float32
BF16 = mybir.dt.bfloat16
FP8 = mybir.dt.float8e4
AF = mybir.ActivationFunctionType
ALU = mybir.AluOpType
DR = mybir.MatmulPerfMode.DoubleRow

B, S, D, H = 8, 1024, 1024, 16
HD = D // H  # 64
INNER = 2730
INNER_P = 2816  # 22*128
P = 128
NT = 8
NKT12 = INNER_P // P  # 22
EPS = 1e-6

SW = 64.0  # weight prescale
SH = 8.0  # hT / h2T prescale
SV = 16.0  # v_sb prescale
SO = 64.0  # ohat prescale
SG = 8.0  # gg prescale (folded into w12 part-1 output)
QS = SW * SH  # 512: psum scale of fp8 matmuls with *8 activations
LN2 = math.log(2.0)

_CACHE = {}
DEBUG = False


def _to_pmaj(v):
    return np.ascontiguousarray(v.reshape(-1, P).T)


def _tileize(w, chunk):
    """[D_in, n*chunk] -> [n, P, (D_in/P)*chunk]: per-output-tile layout that
    is contiguous per partition (fast DMA), matching SBUF [P, kt, chunk]."""
    din, cols = w.shape
    n = cols // chunk
    kt = din // P
    out = np.empty((n, P, kt * chunk), w.dtype)
    for i in range(n):
        blk = w[:, i * chunk : (i + 1) * chunk].reshape(kt, P, chunk)
        out[i] = blk.transpose(1, 0, 2).reshape(P, kt * chunk)
    return np.ascontiguousarray(out)


def _rope_perm():
    ev = np.arange(0, HD, 2)
    od = np.arange(1, HD, 2)
    perm = np.concatenate([ev, od])
    partner = np.concatenate([od, ev])
    return perm, partner


def _q8(x):
    return np.clip(x, -240.0, 240.0).astype(ml_dtypes.float8_e4m3)


def _prep_weights(inp):
    """Host-side layout/dtype/scale prep only (reordering, padding, constant
    scaling by powers of 2 that the device unscales)."""
    perm, partner = _rope_perm()
    chperm = (np.arange(D).reshape(H, HD)[:, perm]).reshape(-1)

    w_qkv, b_qkv = inp["w_qkv"], inp["b_qkv"]
    wq = w_qkv[:, 0:D][:, chperm]
    wk = w_qkv[:, D : 2 * D][:, chperm]
    wv = w_qkv[:, 2 * D :]
    bq = b_qkv[0:D][chperm]
    bk = b_qkv[D : 2 * D][chperm]
    bv = b_qkv[2 * D :]
    wv_ext = np.zeros((D, H * 65), np.float32)
    bv_ext = np.zeros((H * 65,), np.float32)
    for h in range(H):
        wv_ext[:, h * 65 : h * 65 + 64] = wv[:, h * 64 : (h + 1) * 64]
        bv_ext[h * 65 : h * 65 + 64] = bv[h * 64 : (h + 1) * 64] * QS
        bv_ext[h * 65 + 64] = QS  # ones-row for the softmax denominator

    w12, b12 = inp["w12"], inp["b12"]
    w12p = np.zeros((D, 2 * INNER_P), np.float32)
    b12p = np.zeros((2 * INNER_P,), np.float32)
    w12p[:, :INNER] = w12[:, :INNER]
    w12p[:, INNER_P : INNER_P + INNER] = w12[:, INNER:]
    b12p[:INNER] = b12[:INNER]
    b12p[INNER_P : INNER_P + INNER] = b12[INNER:] * SH  # part-1 out is *8
    w3p = np.zeros((INNER_P, D), np.float32)
    w3p[:INNER] = inp["w3"]

    # rope tiles [128, S]: two stacked 64-row head-local blocks
    sign = np.where(np.arange(HD) < HD // 2, -1.0, 1.0).astype(np.float32)
    cos, sin = inp["rope_cos"], inp["rope_sin"]

    def rope_tiles(scale_vec):
        c64 = cos[:, perm].T * scale_vec[perm][:, None]
        s64 = (sin[:, perm].T * sign[:, None]) * scale_vec[partner][:, None]
        return (
            np.concatenate([c64, c64], 0).astype(np.float32),
            np.concatenate([s64, s64], 0).astype(np.float32),
        )

    cq, sq = rope_tiles(inp["qn_scale"])
    ck, sk = rope_tiles(inp["kn_scale"])

    E2 = np.zeros((2, P), np.float32)
    E2[0, 0:64] = 1.0
    E2[1, 64:128] = 1.0
    bo2 = np.zeros((P, 2), np.float32)
    bo2[0:64, 0] = 1.0
    bo2[64:128, 1] = 1.0

    bqk_T = np.stack(
        [bq.reshape(NT, P)[m] for m in range(NT)]
        + [bk.reshape(NT, P)[m] for m in range(NT)],
        axis=1,
    )

    wqk_cat = np.concatenate([wq, wk], axis=1)  # [D, 2048], m-tiles 0..15

    return {
        "wqkT": _q8(_tileize(wqk_cat * SW, P)),  # [16, P, NT*P]
        "wvT": _q8(_tileize(wv_ext * SW, 260)),  # [4, P, NT*260]
        "wprojT": _q8(_tileize(inp["w_proj"] * SW, P)),  # [8, P, NT*P]
        "w12T": _q8(_tileize(w12p * SW, P)),  # [44, P, NT*P]
        "w3T": _q8(_tileize(w3p * SW, P)),  # [8, P, NKT12*P]
        "wadaT": _q8(_tileize(inp["w_ada"] * SW, 512)),  # [12, P, NT*512]
        "bqk_T": bqk_T,
        "bv": bv_ext[None, :].astype(ml_dtypes.bfloat16),
        "b12T": _to_pmaj(b12p),
        "bprojT": _to_pmaj(inp["b_proj"]),
        "b3T": _to_pmaj(inp["b3"]),
        "n1T8": _to_pmaj(inp["norm1_scale"] * SH),
        "n2T8": _to_pmaj(inp["norm2_scale"] * SH),
        "b_ada": inp["b_ada"][None, :],
        "E2": E2.astype(ml_dtypes.bfloat16),
        "bo2": bo2.astype(ml_dtypes.bfloat16),
        "ones1": np.ones((1, P), ml_dtypes.bfloat16),
        "e65": np.concatenate([np.zeros((64, 64), np.float32),
                               np.ones((1, 64), np.float32)]).astype(ml_dtypes.bfloat16),
        "ident": np.eye(P, dtype=np.float32),
        "cos2q": cq.astype(ml_dtypes.bfloat16),
        "sin2q": sq.astype(ml_dtypes.bfloat16),
        "cos2k": ck.astype(ml_dtypes.bfloat16),
        "sin2k": sk.astype(ml_dtypes.bfloat16),
    }


def build_bass():
    nc = bacc.Bacc("TRN2", target_bir_lowering=False, debug=False, num_devices=8)

    def par(name, shape, dt, out=False):
        return nc.declare_dram_parameter(name, list(shape), dt, isOutput=out)

    d = {
        "x": par("x", [S, D], F32),
        "cT": par("cT", [P, NT], F32),
        "wqkT": par("wqkT", [16, P, NT * P], FP8),
        "wvT": par("wvT", [4, P, NT * 260], FP8),
        "wprojT": par("wprojT", [NT, P, NT * P], FP8),
        "w12T": par("w12T", [2 * NKT12, P, NT * P], FP8),
        "w3T": par("w3T", [NT, P, NKT12 * P], FP8),
        "wadaT": par("wadaT", [12, P, NT * 512], FP8),
        "bqk_T": par("bqk_T", [P, 16], F32),
        "bv": par("bv", [1, H * 65], BF16),
        "b12T": par("b12T", [P, 2 * NKT12], F32),
        "bprojT": par("bprojT", [P, NT], F32),
        "b3T": par("b3T", [P, NT], F32),
        "n1T8": par("n1T8", [P, NT], F32),
        "n2T8": par("n2T8", [P, NT], F32),
        "b_ada": par("b_ada", [1, 6 * D], F32),
        "E2": par("E2", [2, P], BF16),
        "bo2": par("bo2", [P, 2], BF16),
        "ones1": par("ones1", [1, P], BF16),
        "e65": par("e65", [65, 64], BF16),
        "ident": par("ident", [P, P], F32),
        "cos2q": par("cos2q", [P, S], BF16),
        "sin2q": par("sin2q", [P, S], BF16),
        "cos2k": par("cos2k", [P, S], BF16),
        "sin2k": par("sin2k", [P, S], BF16),
        "out": par("out", [S, D], F32, out=True),
    }
    if DEBUG:
        for nm, shp, dt in [
            ("dbg_mods", [P, 48], F32), ("dbg_invb", [P, S], F32),
            ("dbg_h0", [P, S], FP8), ("dbg_rawq0", [P, S], BF16),
            ("dbg_rawk0", [P, S], BF16), ("dbg_qhat0", [P, S], BF16),
            ("dbg_khat0", [P, S], BF16), ("dbg_v0", [P, H * 65], FP8),
            ("dbg_pt", [P, NT, 512], FP8), ("dbg_ob", [64, 512], FP8),
            ("dbg_xp0", [P, S], F32), ("dbg_gg0", [P, S], FP8),
        ]:
            d[nm] = par(nm, shp, dt, out=True)
    mods_dram = nc.dram_tensor("mods_scratch", [1, 6 * D], F32)

    with TileContext(nc) as tc:
        _body(nc, tc, d, mods_dram)
    nc.compile()
    return nc


def _body(nc, tc, d, mods_dram):
    from contextlib import ExitStack

    with ExitStack() as ctx:
        const = ctx.enter_context(tc.tile_pool(name="const", bufs=1))
        persist = ctx.enter_context(tc.tile_pool(name="persist", bufs=1))
        small = ctx.enter_context(tc.tile_pool(name="small", bufs=1))

        def load_const(key, shape, dt, pool=None):
            t = (pool or const).tile(list(shape), dt, tag=key, name=key + "_sb")
            nc.sync.dma_start(out=t[:], in_=d[key][:])
            return t

        cT = load_const("cT", [P, NT], F32)
        bqkT = load_const("bqk_T", [P, 16], F32)
        bv = load_const("bv", [1, H * 65], BF16)
        b12T = load_const("b12T", [P, 2 * NKT12], F32)
        bprojT = load_const("bprojT", [P, NT], F32)
        b3T = load_const("b3T", [P, NT], F32)
        n1T8 = load_const("n1T8", [P, NT], F32)
        n2T8 = load_const("n2T8", [P, NT], F32)
        bo2 = load_const("bo2", [P, 2], BF16)
        ones1 = load_const("ones1", [1, P], BF16)
        e65 = load_const("e65", [65, 64], BF16)
        ident = load_const("ident", [P, P], F32)
        E2 = load_const("E2", [2, P], BF16)
        ones128 = const.tile([P, P], BF16, tag="ones128", name="ones128")
        nc.vector.memset(ones128[:], 1.0)
        eps1 = const.tile([P, 1], F32, tag="eps1", name="eps1")
        nc.vector.memset(eps1[:], EPS)
        epsk = const.tile([P, 1], F32, tag="epsk", name="epsk")
        nc.vector.memset(epsk[:], HD * EPS)
        ln2t = const.tile([P, 1], F32, tag="ln2t", name="ln2t")
        nc.vector.memset(ln2t[:], LN2)

        # residual stream lives here, updated in place
        xT = persist.tile([P, NT, S], F32, tag="bigf32", name="xT")
        invb = persist.tile([P, S], F32, tag="invb", name="invb")

        def rms_invb(zT):
            # invb[:, ch*512:...] = 1/sqrt(mean_d z^2 + eps) (rows identical)
            with tc.tile_pool(name="rms_ps", bufs=2, space="PSUM") as rps, \
                 tc.tile_pool(name="rms_sc", bufs=2) as rsc:
                for ch in range(2):
                    ms = None
                    for dt in range(NT):
                        sq = rsc.tile([P, 512], BF16, tag="sqd", name="sqd")
                        eng = nc.vector
                        eng.tensor_mul(
                            sq[:],
                            zT[:, dt, ch * 512 : (ch + 1) * 512],
                            zT[:, dt, ch * 512 : (ch + 1) * 512],
                        )
                        if dt == 0:
                            ms = rps.tile([P, 512], F32, tag="ms", name="ps_ms")
                        nc.tensor.matmul(
                            ms[:], ones128[:], sq[:],
                            start=(dt == 0), stop=(dt == NT - 1),
                        )
                    rms = rsc.tile([P, 512], F32, tag="rms", name="rms")
                    nc.scalar.activation(rms[:], ms[:], AF.Sqrt, bias=eps1[:], scale=1.0 / D)
                    nc.vector.reciprocal_approx_fast(
                        invb[:, ch * 512 : (ch + 1) * 512], rms[:]
                    )

        def modulate(zT, dstT, aa, sh, sc_pool):
            # dstT = 8*(zT*invb*(1+sc) + shift) in fp8 (the *8 is folded into aa/sh)
            for dt in range(NT):
                eng = nc.vector
                tmp = sc_pool.tile([P, S], F32, tag=f"mtmp{dt % 2}", name="mtmp")
                eng.tensor_mul(tmp[:], zT[:, dt, :], invb[:])
                eng.tensor_scalar(
                    dstT[:, dt, :], tmp[:], aa[:, dt : dt + 1], sh[:, dt : dt + 1],
                    op0=ALU.mult, op1=ALU.add,
                )

        # ---- Phase B: load x, transpose to [ch, seq] ----
        with tc.tile_pool(name="xin_pool", bufs=3) as xin_pool, \
             tc.tile_pool(name="psB", bufs=2, space="PSUM") as psB:
            for st in range(NT):
                xin = xin_pool.tile([P, D], F32, tag="xin", name="xin")
                nc.sync.dma_start(out=xin[:], in_=d["x"][st * P : (st + 1) * P, :])
                for g4 in range(2):
                    pt = psB.tile([P, 512], F32, tag="pt", name="ps_tr")
                    for j in range(4):
                        dt = g4 * 4 + j
                        nc.tensor.transpose(
                            pt[:, j * P : (j + 1) * P],
                            xin[:, dt * P : (dt + 1) * P],
                            ident[:],
                        )
                    nc.scalar.activation(
                        xT[:, g4 * 4 : (g4 + 1) * 4, st * P : (st + 1) * P],
                        pt[:], AF.Copy,
                    )

        rms_invb(xT)

        # ---- Phase A: adaLN mods (computed once for both branches) ----
        cT8 = small.tile([P, NT], FP8, name="cT8")
        with tc.tile_pool(name="ada_tmp", bufs=1) as ada_tmp:
            cs = ada_tmp.tile([P, NT], F32, name="cT_silu")
            nc.scalar.activation(cs[:], cT[:], AF.Silu)
            nc.vector.tensor_scalar_mul(cT8[:], cs[:], SH)

        with tc.tile_pool(name="ada_sc", bufs=2) as ada_sc, \
             tc.tile_pool(name="wada_pool", bufs=2) as wada_pool, \
             tc.tile_pool(name="ada_ps", bufs=2, space="PSUM") as ada_ps:
            for n in range(12):
                ps = ada_ps.tile([1, 512], F32, tag="ps", name="ps_ada")
                wt = wada_pool.tile([P, NT, 512], FP8, tag="wada", name="wada_t")
                nc.sync.dma_start(out=wt[:], in_=d["wadaT"][n])
                for kt in range(NT):
                    nc.tensor.matmul(
                        ps[:], cT8[:, kt : kt + 1], wt[:, kt, :],
                        start=(kt == 0), stop=(kt == NT - 1),
                    )
                bch = ada_sc.tile([1, 512], F32, tag="bch", name="bada_ch")
                nc.sync.dma_start(out=bch[:], in_=d["b_ada"][:, n * 512 : (n + 1) * 512])
                mch = ada_sc.tile([1, 512], F32, tag="mch", name="mods_ch")
                nc.vector.scalar_tensor_tensor(
                    mch[:], ps[:], 1.0 / QS, bch[:], op0=ALU.mult, op1=ALU.add
                )
                nc.sync.dma_start(out=mods_dram[:, n * 512 : (n + 1) * 512], in_=mch[:])
        modsT = small.tile([P, 48], F32, name="modsT")
        nc.sync.dma_start(
            out=modsT[:], in_=mods_dram.ap()[0, :].rearrange("(t p) -> p t", p=P)
        )
        if DEBUG:
            nc.sync.dma_start(out=d["dbg_mods"][:], in_=modsT[:])
        # branch-1 (attention) modulation constants, *8 folded for fp8 hT
        a1 = small.tile([P, NT], F32, name="a1")
        nc.vector.tensor_scalar_add(a1[:], modsT[:, 8:16], 1.0)
        nc.vector.tensor_mul(a1[:], a1[:], n1T8[:])
        sh18 = small.tile([P, NT], F32, name="sh18")
        nc.vector.tensor_scalar_mul(sh18[:], modsT[:, 0:8], SH)
        g1 = modsT[:, 16:24]
        g1s = small.tile([P, NT], F32, name="g1s")
        nc.vector.tensor_scalar_mul(g1s[:], g1, 1.0 / (SW * SO))
        g1b = small.tile([P, NT], F32, name="g1b")
        nc.vector.tensor_mul(g1b[:], g1, bprojT[:])
        # branch-2 (MLP)
        a2 = small.tile([P, NT], F32, name="a2")
        nc.vector.tensor_scalar_add(a2[:], modsT[:, 32:40], 1.0)
        nc.vector.tensor_mul(a2[:], a2[:], n2T8[:])
        sh28 = small.tile([P, NT], F32, name="sh28")
        nc.vector.tensor_scalar_mul(sh28[:], modsT[:, 24:32], SH)
        g2 = modsT[:, 40:48]
        g2s = small.tile([P, NT], F32, name="g2s")
        nc.vector.tensor_scalar_mul(g2s[:], g2, 1.0 / (SW * SG))
        g2b3 = small.tile([P, NT], F32, name="g2b3")
        nc.vector.tensor_mul(g2b3[:], g2, b3T[:])

        # ======= attention branch =======
        with ExitStack() as actx:
            ho = actx.enter_context(tc.tile_pool(name="ho", bufs=1))
            ohat = ho.tile([P, NT, S], FP8, tag="ohat", name="ohat")

            with ExitStack() as cctx:
                qk = cctx.enter_context(tc.tile_pool(name="qk", bufs=1))
                qhat = qk.tile([P, NT, S], BF16, tag="qhat", name="qhat")
                khat = qk.tile([P, NT, S], BF16, tag="khat", name="khat")
                v_sb = qk.tile([P, NT, H * 65], FP8, tag="v", name="v_sb")

                hop = cctx.enter_context(tc.tile_pool(name="hop", bufs=1))
                hT = hop.tile([P, NT, S], FP8, tag="hT", name="hT")

                with ExitStack() as qctx:
                    mod_sc = qctx.enter_context(tc.tile_pool(name="mod_sc", bufs=2))
                    modulate(xT, hT, a1, sh18, mod_sc)
                    if DEBUG:
                        nc.sync.dma_start(out=d["dbg_invb"][:], in_=invb[:])
                        nc.sync.dma_start(out=d["dbg_h0"][:], in_=hT[:, 0, :])

                    ropec = qctx.enter_context(tc.tile_pool(name="ropec", bufs=1))
                    qkn = qctx.enter_context(tc.tile_pool(name="qkn", bufs=1))
                    wqk_pool = qctx.enter_context(tc.tile_pool(name="wqk_pool", bufs=3))
                    rope_sc = qctx.enter_context(tc.tile_pool(name="rope_sc", bufs=2))
                    qk_ps = qctx.enter_context(
                        tc.tile_pool(name="qk_ps", bufs=3, space="PSUM")
                    )
                    pe_ps = qctx.enter_context(
                        tc.tile_pool(name="pe_ps", bufs=2, space="PSUM")
                    )
                    ss_ps = qctx.enter_context(
                        tc.tile_pool(name="ss_ps", bufs=2, space="PSUM")
                    )

                    cos2q = load_const("cos2q", [P, S], BF16, pool=ropec)
                    sin2q = load_const("sin2q", [P, S], BF16, pool=ropec)
                    cos2k = load_const("cos2k", [P, S], BF16, pool=ropec)
                    sin2k = load_const("sin2k", [P, S], BF16, pool=ropec)

                    # k tiles first (m 8..15), then q tiles (m 0..7)
                    for m in list(range(NT, 2 * NT)) + list(range(NT)):
                        isq = m < NT
                        mk = m if isq else m - NT
                        wt = wqk_pool.tile([P, NT, P], FP8, tag="wqk", name="wqk_t")
                        nc.sync.dma_start(out=wt[:], in_=d["wqkT"][m])
                        raw = rope_sc.tile([P, S], BF16, tag="raw", name="qk_raw")
                        stage = qkn.tile([2, S], F32, tag="stage", name="stage", bufs=2)
                        for sch in range(2):
                            ps = qk_ps.tile([P, 512], F32, tag="ps", name="ps_qkv")
                            for t in range(4):
                                nc.tensor.matmul(
                                    ps[:], wt[:, 2 * t : 2 * t + 2, :],
                                    hT[:, 2 * t : 2 * t + 2, sch * 512 : (sch + 1) * 512],
                                    start=(t == 0), stop=(t == 3), perf_mode=DR,
                                )
                            if sch == 0:
                                nc.scalar.activation(
                                    raw[:, sch * 512 : (sch + 1) * 512], ps[:],
                                    AF.Identity,
                                    bias=bqkT[:, m : m + 1], scale=1.0 / QS,
                                )
                            else:
                                nc.vector.tensor_scalar(
                                    raw[:, sch * 512 : (sch + 1) * 512], ps[:],
                                    1.0 / QS, bqkT[:, m : m + 1],
                                    op0=ALU.mult, op1=ALU.add,
                                )
                            sqs = rope_sc.tile([P, 512], BF16, tag="sqs", name="sqs")
                            if sch == 0:
                                nc.scalar.activation(
                                    sqs[:], raw[:, sch * 512 : (sch + 1) * 512],
                                    AF.Square,
                                )
                            else:
                                nc.vector.tensor_mul(
                                    sqs[:],
                                    raw[:, sch * 512 : (sch + 1) * 512],
                                    raw[:, sch * 512 : (sch + 1) * 512],
                                )
                            ss = ss_ps.tile([2, 512], F32, tag="ss", name="ps_ss")
                            nc.tensor.matmul(ss[:], bo2[:], sqs[:], start=True, stop=True)
                            nc.scalar.activation(
                                stage[:, sch * 512 : (sch + 1) * 512], ss[:], AF.Sqrt,
                                bias=(eps1[0:2, :] if isq else epsk[0:2, :]),
                                scale=(1.0 / HD if isq else 1.0),
                            )
                        # rope: rotate-half via 32-row block swaps
                        rot = rope_sc.tile([P, S], BF16, tag="rot", name="rot", bufs=2)
                        for blk in range(4):
                            b0 = blk * 32
                            srcb = b0 + (32 if blk % 2 == 0 else -32)
                            nc.gpsimd.dma_start(
                                out=rot[b0 : b0 + 32, :], in_=raw[srcb : srcb + 32, :]
                            )
                        t1 = rope_sc.tile([P, S], BF16, tag="t1", name="rope_t1", bufs=2)
                        t2 = rope_sc.tile([P, S], BF16, tag="t2", name="rope_t2", bufs=2)
                        u = rope_sc.tile([P, S], BF16, tag="u", name="rope_u", bufs=2)
                        nc.vector.tensor_mul(t1[:], raw[:], cos2q[:] if isq else cos2k[:])
                        nc.vector.tensor_mul(t2[:], rot[:], sin2q[:] if isq else sin2k[:])
                        nc.vector.tensor_add(u[:], t1[:], t2[:])
                        # fold inverse-rms (q) / inverse-rms/8 (k) into the tile
                        rstage = qkn.tile([2, S], F32, tag="rstage", name="rstage", bufs=2)
                        nc.vector.reciprocal_approx_fast(rstage[:], stage[:])
                        rbf = qkn.tile([2, S], BF16, tag="rbf", name="rbf", bufs=2)
                        nc.scalar.activation(rbf[:], rstage[:], AF.Copy)
                        dst = qhat if isq else khat
                        for sch in range(2):
                            pe = pe_ps.tile([P, 512], F32, tag="pe", name="ps_erq")
                            nc.tensor.matmul(
                                pe[:], E2[:], rbf[:, sch * 512 : (sch + 1) * 512],
                                start=True, stop=True,
                            )
                            nc.vector.tensor_mul(
                                dst[:, mk, sch * 512 : (sch + 1) * 512],
                                u[:, sch * 512 : (sch + 1) * 512], pe[:],
                            )
                        if DEBUG and mk == 0:
                            nc.sync.dma_start(
                                out=d["dbg_rawq0" if isq else "dbg_rawk0"][:],
                                in_=raw[:],
                            )
                            nc.sync.dma_start(
                                out=d["dbg_qhat0" if isq else "dbg_khat0"][:],
                                in_=dst[:, 0, :],
                            )

                # v (fp8, *16; bias row gives the softmax denominator)
                with tc.tile_pool(name="wv_pool", bufs=2) as wv_pool, \
                     tc.tile_pool(name="v_ps", bufs=3, space="PSUM") as v_ps:
                    for nch in range(4):
                        c0 = nch * 260
                        wt = wv_pool.tile([P, NT, 260], FP8, tag="wv", name="wv_t")
                        nc.sync.dma_start(out=wt[:], in_=d["wvT"][nch])
                        for st in range(NT):
                            ps = v_ps.tile([P, 260], F32, tag="ps", name="ps_v")
                            for t in range(4):
                                nc.tensor.matmul(
                                    ps[:],
                                    hT[:, 2 * t : 2 * t + 2, st * P : (st + 1) * P],
                                    wt[:, 2 * t : 2 * t + 2, :],
                                    start=(t == 0), stop=False, perf_mode=DR,
                                )
                            nc.tensor.matmul(
                                ps[:], ones1[:], bv[:, c0 : c0 + 260],
                                start=False, stop=True, skip_group_check=True,
                            )
                            nc.vector.tensor_scalar_mul(
                                v_sb[:, st, c0 : c0 + 260], ps[:], SV / QS
                            )
                    if DEBUG:
                        nc.sync.dma_start(out=d["dbg_v0"][:], in_=v_sb[:, 0, :])

                # ---- attention + pipelined MLP half 0 ----
                mlp = cctx.enter_context(tc.tile_pool(name="mlp", bufs=1))
                h2T = mlp.tile([P, NT, S], FP8, tag="h2T", name="h2T")
                gg = mlp.tile([P, NKT12, S], FP8, tag="gg", name="gg")
                wproj_pool = cctx.enter_context(tc.tile_pool(name="wproj_pool", bufs=1))
                wp_tiles = []
                for dt in range(NT):
                    wt = wproj_pool.tile([P, NT, P], FP8, tag=f"wp{dt}", name=f"wp{dt}")
                    nc.sync.dma_start(out=wt[:], in_=d["wprojT"][dt])
                    wp_tiles.append(wt)

                with tc.tile_pool(name="psA", bufs=1, space="PSUM") as psA, \
                     tc.tile_pool(name="avpb", bufs=1, space="PSUM") as avpb, \
                     tc.tile_pool(name="mlpC", bufs=3, space="PSUM") as mlpC, \
                     tc.tile_pool(name="pT_pool", bufs=2) as pT_pool, \
                     tc.tile_pool(name="att_sc", bufs=2) as att_sc, \
                     tc.tile_pool(name="mod_sc2", bufs=2) as mod_sc2, \
                     tc.tile_pool(name="w12_pool", bufs=3) as w12_pool, \
                     tc.tile_pool(name="mlp_sc", bufs=2) as mlp_sc:

                    def qk_exp(qch, h):
                        mk, hh = h // 2, h % 2
                        rb = 64 * hh
                        pT = pT_pool.tile([P, NT, 512], FP8, tag="pT", name="pT")
                        for g in range(2):
                            grp = psA.tile([P, 4, 512], F32, tag="g4", name="ps_g")
                            for j in range(4):
                                kt = g * 4 + j
                                nc.tensor.matmul(
                                    grp[:, j, :],
                                    khat[rb : rb + 64, mk, kt * P : (kt + 1) * P],
                                    qhat[rb : rb + 64, mk, qch * 512 : (qch + 1) * 512],
                                    start=True, stop=True,
                                )
                            nc.scalar.activation(
                                pT[:, g * 4 : (g + 1) * 4, :], grp[:], AF.Exp,
                                bias=0.0, scale=1.0,
                            )
                        return pT

                    def av_div(qch, h, pT):
                        mk, hh = h // 2, h % 2
                        rb = 64 * hh
                        av = avpb.tile([65, 512], F32, tag="avpb", name="ps_av")
                        for t in range(4):
                            nc.tensor.matmul(
                                av[:], v_sb[:, 2 * t : 2 * t + 2, h * 65 : h * 65 + 65],
                                pT[:, 2 * t : 2 * t + 2, :],
                                start=(t == 0), stop=(t == 3), perf_mode=DR,
                            )
                        o65b = att_sc.tile([65, 512], BF16, tag="o65b", name="o65b")
                        nc.vector.tensor_copy(o65b[:], av[:])
                        pb = avpb.tile([65, 512], F32, tag="avpb", name="ps_pb")
                        nc.tensor.matmul(pb[0:64, :], e65[:], o65b[:], start=True, stop=True)
                        rb64 = att_sc.tile([64, 512], F32, tag="rb64", name="rb64")
                        nc.vector.reciprocal_approx_fast(rb64[:], pb[0:64, :])
                        ob = att_sc.tile([64, 512], FP8, tag="ob", name="ob")
                        nc.vector.scalar_tensor_tensor(
                            ob[:], o65b[0:64, :], SO, rb64[:], op0=ALU.mult, op1=ALU.mult
                        )
                        nc.sync.dma_start(
                            out=ohat[rb : rb + 64, mk, qch * 512 : (qch + 1) * 512],
                            in_=ob[:],
                        )
                        if DEBUG and qch == 0 and h == 0:
                            nc.sync.dma_start(out=d["dbg_pt"][:], in_=pT[:])
                            nc.sync.dma_start(out=d["dbg_ob"][:], in_=ob[:])

                    # MLP half-0 work emitted between attention qch=1 heads so
                    # its PE work fills the exp-bound gaps
                    def mk_proj0(dt):
                        def f():
                            ps = mlpC.tile([P, 512], F32, tag="c", name="ps_c")
                            for t in range(4):
                                nc.tensor.matmul(
                                    ps[:], wp_tiles[dt][:, 2 * t : 2 * t + 2, :],
                                    ohat[:, 2 * t : 2 * t + 2, 0:512],
                                    start=(t == 0), stop=(t == 3), perf_mode=DR,
                                )
                            nc.vector.affine_then_add(
                                xT[:, dt, 0:512], ps[:], xT[:, dt, 0:512],
                                scale=g1s[:, dt : dt + 1], bias=g1b[:, dt : dt + 1],
                            )
                        return f

                    def mk_rms2(ch):
                        def f():
                            ms = mlpC.tile([P, 512], F32, tag="c", name="ps_c")
                            for dt in range(NT):
                                sq = mod_sc2.tile([P, 512], BF16, tag="sq2", name="sq2")
                                nc.vector.tensor_mul(
                                    sq[:], xT[:, dt, ch * 512 : (ch + 1) * 512],
                                    xT[:, dt, ch * 512 : (ch + 1) * 512],
                                )
                                nc.tensor.matmul(
                                    ms[:], ones128[:], sq[:],
                                    start=(dt == 0), stop=(dt == NT - 1),
                                )
                            rms = mod_sc2.tile([P, 512], F32, tag="rms2", name="rms2")
                            nc.scalar.activation(
                                rms[:], ms[:], AF.Sqrt, bias=eps1[:], scale=1.0 / D
                            )
                            nc.vector.reciprocal_approx_fast(
                                invb[:, ch * 512 : (ch + 1) * 512], rms[:]
                            )
                        return f

                    def mk_mod2(dts, ch):
                        def f():
                            for dt in dts:
                                tmp = mod_sc2.tile([P, 512], F32, tag="m2tmp", name="m2tmp")
                                nc.gpsimd.tensor_mul(
                                    tmp[:], xT[:, dt, ch * 512 : (ch + 1) * 512],
                                    invb[:, ch * 512 : (ch + 1) * 512],
                                )
                                nc.vector.tensor_scalar(
                                    h2T[:, dt, ch * 512 : (ch + 1) * 512], tmp[:],
                                    a2[:, dt : dt + 1], sh28[:, dt : dt + 1],
                                    op0=ALU.mult, op1=ALU.add,
                                )
                        return f

                    def mk_w12(j, sch):
                        def f():
                            outs = []
                            for part in range(2):
                                wt = w12_pool.tile([P, NT, P], FP8, tag="w12", name="w12_t")
                                nc.sync.dma_start(out=wt[:], in_=d["w12T"][part * NKT12 + j])
                                ps = mlpC.tile([P, 512], F32, tag="c", name="ps_c")
                                for t in range(4):
                                    nc.tensor.matmul(
                                        ps[:], wt[:, 2 * t : 2 * t + 2, :],
                                        h2T[:, 2 * t : 2 * t + 2, sch * 512 : (sch + 1) * 512],
                                        start=(t == 0), stop=(t == 3), perf_mode=DR,
                                    )
                                o = mlp_sc.tile(
                                    [P, 512], BF16, tag=f"mlp{part}", name=f"mlp{part}"
                                )
                                if part == 0:
                                    nc.scalar.activation(
                                        o[:], ps[:], AF.Silu,
                                        bias=b12T[:, j : j + 1], scale=1.0 / QS,
                                    )
                                else:
                                    nc.vector.tensor_scalar(
                                        o[:], ps[:], SG / QS,
                                        b12T[:, NKT12 + j : NKT12 + j + 1],
                                        op0=ALU.mult, op1=ALU.add,
                                    )
                                outs.append(o)
                            nc.gpsimd.tensor_mul(
                                gg[:, j, sch * 512 : (sch + 1) * 512],
                                outs[0][:], outs[1][:],
                            )
                        return f

                    work0 = (
                        [mk_proj0(dt) for dt in range(NT)]
                        + [mk_rms2(0)]
                        + [mk_mod2(range(4 * i, 4 * i + 4), 0) for i in range(2)]
                        + [mk_w12(j, 0) for j in range(NKT12)]
                    )

                    prev = None
                    for qch in range(2):
                        for h in range(H):
                            if prev is not None:
                                av_div(*prev)
                            if qch == 1 and work0:
                                work0.pop(0)()
                                if work0:
                                    work0.pop(0)()
                            pT = qk_exp(qch, h)
                            prev = (qch, h, pT)
                    av_div(*prev)
                    for f in work0:
                        f()

                # ---- tail: proj half 1, MLP half 1, w3, output ----
                with tc.tile_pool(name="tail_ps", bufs=3, space="PSUM") as tail_ps, \
                     tc.tile_pool(name="tail_sc", bufs=2) as tail_sc, \
                     tc.tile_pool(name="w12b_pool", bufs=3) as w12b_pool:
                    for dt in range(NT):
                        ps = tail_ps.tile([P, 512], F32, tag="t", name="ps_t")
                        for t in range(4):
                            nc.tensor.matmul(
                                ps[:], wp_tiles[dt][:, 2 * t : 2 * t + 2, :],
                                ohat[:, 2 * t : 2 * t + 2, 512:1024],
                                start=(t == 0), stop=(t == 3), perf_mode=DR,
                            )
                        nc.vector.affine_then_add(
                            xT[:, dt, 512:1024], ps[:], xT[:, dt, 512:1024],
                            scale=g1s[:, dt : dt + 1], bias=g1b[:, dt : dt + 1],
                        )
                    ms = tail_ps.tile([P, 512], F32, tag="t", name="ps_t")
                    for dt in range(NT):
                        sq = tail_sc.tile([P, 512], BF16, tag="sq", name="sq")
                        nc.vector.tensor_mul(
                            sq[:], xT[:, dt, 512:1024], xT[:, dt, 512:1024]
                        )
                        nc.tensor.matmul(
                            ms[:], ones128[:], sq[:],
                            start=(dt == 0), stop=(dt == NT - 1),
                        )
                    rms = tail_sc.tile([P, 512], F32, tag="rms", name="rms")
                    nc.scalar.activation(rms[:], ms[:], AF.Sqrt, bias=eps1[:], scale=1.0 / D)
                    nc.vector.reciprocal_approx_fast(invb[:, 512:1024], rms[:])
                    for dt in range(NT):
                        tmp = tail_sc.tile([P, 512], F32, tag="m2", name="m2")
                        eng = nc.gpsimd if dt % 2 else nc.vector
                        eng.tensor_mul(tmp[:], xT[:, dt, 512:1024], invb[:, 512:1024])
                        nc.vector.tensor_scalar(
                            h2T[:, dt, 512:1024], tmp[:],
                            a2[:, dt : dt + 1], sh28[:, dt : dt + 1],
                            op0=ALU.mult, op1=ALU.add,
                        )
                    for j in range(NKT12):
                        outs = []
                        for part in range(2):
                            wt = w12b_pool.tile([P, NT, P], FP8, tag="w12b", name="w12b_t")
                            nc.sync.dma_start(out=wt[:], in_=d["w12T"][part * NKT12 + j])
                            ps = tail_ps.tile([P, 512], F32, tag="t", name="ps_t")
                            for t in range(4):
                                nc.tensor.matmul(
                                    ps[:], wt[:, 2 * t : 2 * t + 2, :],
                                    h2T[:, 2 * t : 2 * t + 2, 512:1024],
                                    start=(t == 0), stop=(t == 3), perf_mode=DR,
                                )
                            o = tail_sc.tile([P, 512], BF16, tag=f"o{part}", name=f"o{part}")
                            if part == 0:
                                nc.scalar.activation(
                                    o[:], ps[:], AF.Silu,
                                    bias=b12T[:, j : j + 1], scale=1.0 / QS,
                                )
                            else:
                                nc.vector.tensor_scalar(
                                    o[:], ps[:], SG / QS,
                                    b12T[:, NKT12 + j : NKT12 + j + 1],
                                    op0=ALU.mult, op1=ALU.add,
                                )
                            outs.append(o)
                        nc.gpsimd.tensor_mul(
                            gg[:, j, 512:1024], outs[0][:], outs[1][:]
                        )
                    if DEBUG:
                        nc.sync.dma_start(out=d["dbg_xp0"][:], in_=xT[:, 0, :])
                        nc.sync.dma_start(out=d["dbg_gg0"][:], in_=gg[:, 0, :])
                    with tc.tile_pool(name="w3_pool", bufs=2) as w3_pool:
                        for dt in range(NT):
                            wt = w3_pool.tile([P, NKT12, P], FP8, tag="w3", name="w3_t")
                            nc.sync.dma_start(out=wt[:], in_=d["w3T"][dt])
                            for qch in range(2):
                                ps = tail_ps.tile([P, 512], F32, tag="t", name="ps_t")
                                for t in range(11):
                                    nc.tensor.matmul(
                                        ps[:], wt[:, 2 * t : 2 * t + 2, :],
                                        gg[:, 2 * t : 2 * t + 2, qch * 512 : (qch + 1) * 512],
                                        start=(t == 0), stop=(t == 10), perf_mode=DR,
                                    )
                                nc.vector.affine_then_add(
                                    xT[:, dt, qch * 512 : (qch + 1) * 512], ps[:],
                                    xT[:, dt, qch * 512 : (qch + 1) * 512],
                                    scale=g2s[:, dt : dt + 1], bias=g2b3[:, dt : dt + 1],
                                )
                    with tc.tile_pool(name="yout", bufs=3) as ypool, \
                         tc.tile_pool(name="psH", bufs=2, space="PSUM") as psH:
                        for st in range(NT):
                            y = ypool.tile([P, D], F32, tag="y", name="y")
                            for g4 in range(2):
                                pt = psH.tile([P, 512], F32, tag="pt", name="ps_tr2")
                                for j in range(4):
                                    dt = g4 * 4 + j
                                    nc.tensor.transpose(
                                        pt[:, j * P : (j + 1) * P],
                                        xT[:, dt, st * P : (st + 1) * P],
                                        ident[:],
                                    )
                                nc.scalar.activation(
                                    y[:, g4 * 512 : (g4 + 1) * 512], pt[:], AF.Copy
                                )
                            nc.sync.dma_start(
                                out=d["out"][st * P : (st + 1) * P, :], in_=y[:]
                            )


def kernel(**inputs):
    inputs = {k: np.asarray(v) for k, v in inputs.items()}
    if "nc" not in _CACHE:
        _CACHE["nc"] = build_bass()
    nc = _CACHE["nc"]

    base = _prep_weights(inputs)

    in_maps = []
    for core in range(B):
        m = dict(base)
        m["x"] = np.ascontiguousarray(inputs["x"][core]).astype(np.float32)
        m["cT"] = _to_pmaj(inputs["c"][core]).astype(np.float32)
        in_maps.append(m)

    res = run_bass_kernel_spmd(
        nc, in_maps, core_ids=list(range(B)), **_CACHE.get("run_kwargs", {})
    )
    _CACHE["last_results"] = res
    return np.stack([res.results[i]["out"] for i in range(B)], axis=0)


if __name__ == "__main__":
    build_bass()
    print("built ok")


# revision 18
# speedup vs baseline: 1.2301x; 1.2301x over previous
"""JiT/DiT transformer block (adaLN + attention + SwiGLU) on 8 TRN2 NeuronCores.

Data-parallel over batch: core i computes batch element i end-to-end; no
collectives. Activations kept "transposed" on device ([channel, seq]) so
per-channel modulation/bias are per-partition scalars; attention scores are
produced directly in [k, q] layout (softmax denominator via a ones-row
appended to V inside the AV matmul).

All big linear matmuls (qkv, v, proj, w12, w3, ada, AV) run fp8e4 with
MatmulPerfMode.DoubleRow (2 contraction k-tiles per instruction, 2x rate),
fp32 PSUM accumulation. Weights are host-prescaled by 64, activations by a
power-of-2 per tensor; the products are unscaled on the PSUM copy-out (the
combined factor folds into the existing scale/bias of that op). Scores stay
bf16. The residual stream stays fp32.

Scale conventions (host ``*`` prescale / device unscale):
  weights *64 | hT,h2T *8 | v_sb *16 (bias row = 16) | ohat *64 | gg *8
  qkv psum = 512*qk -> raw = ps/512 + b
  exp bias +ln2 (cancels in softmax ratio, keeps pT in fp8 normal range)
"""

import sys

sys.path.insert(0, "/opt/trn_rl_repo")

import math

import numpy as np
import ml_dtypes

import concourse.bacc as bacc
import concourse.bass as bass
import concourse.mybir as mybir
from concourse.tile import TileContext
from concourse.bass_utils import run_bass_kernel_spmd

F32 = mybir.dt.float32
BF16 = mybir.dt.bfloat16
FP8 = mybir.dt.float8e4
AF = mybir.ActivationFunctionType
ALU = mybir.AluOpType
DR = mybir.MatmulPerfMode.DoubleRow

B, S, D, H = 8, 1024, 1024, 16
HD = D // H  # 64
INNER = 2730
INNER_P = 2816  # 22*128
P = 128
NT = 8
NKT12 = INNER_P // P  # 22
EPS = 1e-6

SW = 64.0  # weight prescale
SH = 8.0  # hT / h2T prescale
SV = 16.0  # v_sb prescale
SO = 64.0  # ohat prescale
SG = 8.0  # gg prescale (folded into w12 part-1 output)
QS = SW * SH  # 512: psum scale of fp8 matmuls with *8 activations
LN2 = math.log(2.0)

_CACHE = {}
DEBUG = False


def _to_pmaj(v):
    return np.ascontiguousarray(v.reshape(-1, P).T)


def _tileize(w, chunk):
    """[D_in, n*chunk] -> [n, P, (D_in/P)*chunk]: per-output-tile layout that
    is contiguous per partition (fast DMA), matching SBUF [P, kt, chunk]."""
    din, cols = w.shape
    n = cols // chunk
    kt = din // P
    out = np.empty((n, P, kt * chunk), w.dtype)
    for i in range(n):
        blk = w[:, i * chunk : (i + 1) * chunk].reshape(kt, P, chunk)
        out[i] = blk.transpose(1, 0, 2).reshape(P, kt * chunk)
    return np.ascontiguousarray(out)


def _rope_perm():
    ev = np.arange(0, HD, 2)
    od = np.arange(1, HD, 2)
    perm = np.concatenate([ev, od])
    partner = np.concatenate([od, ev])
    return perm, partner


def _q8(x):
    return np.clip(x, -240.0, 240.0).astype(ml_dtypes.float8_e4m3)


def _prep_weights(inp):
    """Host-side layout/dtype/scale prep only (reordering, padding, constant
    scaling by powers of 2 that the device unscales)."""
    perm, partner = _rope_perm()
    chperm = (np.arange(D).reshape(H, HD)[:, perm]).reshape(-1)

    w_qkv, b_qkv = inp["w_qkv"], inp["b_qkv"]
    wq = w_qkv[:, 0:D][:, chperm]
    wk = w_qkv[:, D : 2 * D][:, chperm]
    wv = w_qkv[:, 2 * D :]
    bq = b_qkv[0:D][chperm]
    bk = b_qkv[D : 2 * D][chperm]
    bv = b_qkv[2 * D :]
    wv_ext = np.zeros((D, H * 65), np.float32)
    bv_ext = np.zeros((H * 65,), np.float32)
    for h in range(H):
        wv_ext[:, h * 65 : h * 65 + 64] = wv[:, h * 64 : (h + 1) * 64]
        bv_ext[h * 65 : h * 65 + 64] = bv[h * 64 : (h + 1) * 64] * QS
        bv_ext[h * 65 + 64] = QS  # ones-row for the softmax denominator

    w12, b12 = inp["w12"], inp["b12"]
    w12p = np.zeros((D, 2 * INNER_P), np.float32)
    b12p = np.zeros((2 * INNER_P,), np.float32)
    w12p[:, :INNER] = w12[:, :INNER]
    w12p[:, INNER_P : INNER_P + INNER] = w12[:, INNER:]
    b12p[:INNER] = b12[:INNER]
    b12p[INNER_P : INNER_P + INNER] = b12[INNER:] * SH  # part-1 out is *8
    w3p = np.zeros((INNER_P, D), np.float32)
    w3p[:INNER] = inp["w3"]

    # rope tiles [128, S]: two stacked 64-row head-local blocks
    sign = np.where(np.arange(HD) < HD // 2, -1.0, 1.0).astype(np.float32)
    cos, sin = inp["rope_cos"], inp["rope_sin"]

    def rope_tiles(scale_vec):
        c64 = cos[:, perm].T * scale_vec[perm][:, None]
        s64 = (sin[:, perm].T * sign[:, None]) * scale_vec[partner][:, None]
        return (
            np.concatenate([c64, c64], 0).astype(np.float32),
            np.concatenate([s64, s64], 0).astype(np.float32),
        )

    cq, sq = rope_tiles(inp["qn_scale"])
    ck, sk = rope_tiles(inp["kn_scale"])

    E2 = np.zeros((2, P), np.float32)
    E2[0, 0:64] = 1.0
    E2[1, 64:128] = 1.0
    bo2 = np.zeros((P, 2), np.float32)
    bo2[0:64, 0] = 1.0
    bo2[64:128, 1] = 1.0

    bqk_T = np.stack(
        [bq.reshape(NT, P)[m] for m in range(NT)]
        + [bk.reshape(NT, P)[m] for m in range(NT)],
        axis=1,
    )

    wqk_cat = np.concatenate([wq, wk], axis=1)  # [D, 2048], m-tiles 0..15

    return {
        "wqkT": _q8(_tileize(wqk_cat * SW, P)),  # [16, P, NT*P]
        "wvT": _q8(_tileize(wv_ext * SW, 260)),  # [4, P, NT*260]
        "wprojT": _q8(_tileize(inp["w_proj"] * SW, P)),  # [8, P, NT*P]
        "w12T": _q8(_tileize(w12p * SW, P)),  # [44, P, NT*P]
        "w3T": _q8(_tileize(w3p * SW, P)),  # [8, P, NKT12*P]
        "wadaT": _q8(_tileize(inp["w_ada"] * SW, 512)),  # [12, P, NT*512]
        "bqk_T": bqk_T,
        "bv": bv_ext[None, :].astype(ml_dtypes.bfloat16),
        "b12T": _to_pmaj(b12p),
        "bprojT": _to_pmaj(inp["b_proj"]),
        "b3T": _to_pmaj(inp["b3"]),
        "n1T8": _to_pmaj(inp["norm1_scale"] * SH),
        "n2T8": _to_pmaj(inp["norm2_scale"] * SH),
        "b_ada": inp["b_ada"][None, :],
        "E2": E2.astype(ml_dtypes.bfloat16),
        "bo2": bo2.astype(ml_dtypes.bfloat16),
        "ones1": np.ones((1, P), ml_dtypes.bfloat16),
        "e65": np.concatenate([np.zeros((64, 64), np.float32),
                               np.ones((1, 64), np.float32)]).astype(ml_dtypes.bfloat16),
        "ident": np.eye(P, dtype=np.float32),
        "cos2q": cq.astype(ml_dtypes.bfloat16),
        "sin2q": sq.astype(ml_dtypes.bfloat16),
        "cos2k": ck.astype(ml_dtypes.bfloat16),
        "sin2k": sk.astype(ml_dtypes.bfloat16),
    }


def build_bass():
    nc = bacc.Bacc("TRN2", target_bir_lowering=False, debug=False, num_devices=8)

    def par(name, shape, dt, out=False):
        return nc.declare_dram_parameter(name, list(shape), dt, isOutput=out)

    d = {
        "x": par("x", [S, D], F32),
        "cT": par("cT", [P, NT], F32),
        "wqkT": par("wqkT", [16, P, NT * P], FP8),
        "wvT": par("wvT", [4, P, NT * 260], FP8),
        "wprojT": par("wprojT", [NT, P, NT * P], FP8),
        "w12T": par("w12T", [2 * NKT12, P, NT * P], FP8),
        "w3T": par("w3T", [NT, P, NKT12 * P], FP8),
        "wadaT": par("wadaT", [12, P, NT * 512], FP8),
        "bqk_T": par("bqk_T", [P, 16], F32),
        "bv": par("bv", [1, H * 65], BF16),
        "b12T": par("b12T", [P, 2 * NKT12], F32),
        "bprojT": par("bprojT", [P, NT], F32),
        "b3T": par("b3T", [P, NT], F32),
        "n1T8": par("n1T8", [P, NT], F32),
        "n2T8": par("n2T8", [P, NT], F32),
        "b_ada": par("b_ada", [1, 6 * D], F32),
        "E2": par("E2", [2, P], BF16),
        "bo2": par("bo2", [P, 2], BF16),
        "ones1": par("ones1", [1, P], BF16),
        "e65": par("e65", [65, 64], BF16),
        "ident": par("ident", [P, P], F32),
        "cos2q": par("cos2q", [P, S], BF16),
        "sin2q": par("sin2q", [P, S], BF16),
        "cos2k": par("cos2k", [P, S], BF16),
        "sin2k": par("sin2k", [P, S], BF16),
        "out": par("out", [S, D], F32, out=True),
    }
    if DEBUG:
        for nm, shp, dt in [
            ("dbg_mods", [P, 48], F32), ("dbg_invb", [P, S], F32),
            ("dbg_h0", [P, S], FP8), ("dbg_rawq0", [P, S], BF16),
            ("dbg_rawk0", [P, S], BF16), ("dbg_qhat0", [P, S], BF16),
            ("dbg_khat0", [P, S], BF16), ("dbg_v0", [P, H * 65], FP8),
            ("dbg_pt", [P, NT, 512], FP8), ("dbg_ob", [64, 512], FP8),
            ("dbg_xp0", [P, S], F32), ("dbg_gg0", [P, S], FP8),
        ]:
            d[nm] = par(nm, shp, dt, out=True)
    mods_dram = nc.dram_tensor("mods_scratch", [1, 6 * D], F32)

    with TileContext(nc) as tc:
        _body(nc, tc, d, mods_dram)
    nc.compile()
    return nc


def _body(nc, tc, d, mods_dram):
    from contextlib import ExitStack

    with ExitStack() as ctx:
        const = ctx.enter_context(tc.tile_pool(name="const", bufs=1))
        persist = ctx.enter_context(tc.tile_pool(name="persist", bufs=1))
        small = ctx.enter_context(tc.tile_pool(name="small", bufs=1))

        def load_const(key, shape, dt, pool=None):
            t = (pool or const).tile(list(shape), dt, tag=key, name=key + "_sb")
            nc.sync.dma_start(out=t[:], in_=d[key][:])
            return t

        cT = load_const("cT", [P, NT], F32)
        bqkT = load_const("bqk_T", [P, 16], F32)
        bv = load_const("bv", [1, H * 65], BF16)
        b12T = load_const("b12T", [P, 2 * NKT12], F32)
        bprojT = load_const("bprojT", [P, NT], F32)
        b3T = load_const("b3T", [P, NT], F32)
        n1T8 = load_const("n1T8", [P, NT], F32)
        n2T8 = load_const("n2T8", [P, NT], F32)
        bo2 = load_const("bo2", [P, 2], BF16)
        ones1 = load_const("ones1", [1, P], BF16)
        e65 = load_const("e65", [65, 64], BF16)
        ident = load_const("ident", [P, P], F32)
        E2 = load_const("E2", [2, P], BF16)
        ones128 = const.tile([P, P], BF16, tag="ones128", name="ones128")
        nc.vector.memset(ones128[:], 1.0)
        eps1 = const.tile([P, 1], F32, tag="eps1", name="eps1")
        nc.vector.memset(eps1[:], EPS)
        epsk = const.tile([P, 1], F32, tag="epsk", name="epsk")
        nc.vector.memset(epsk[:], HD * EPS)
        ln2t = const.tile([P, 1], F32, tag="ln2t", name="ln2t")
        nc.vector.memset(ln2t[:], LN2)

        # residual stream lives here, updated in place
        xT = persist.tile([P, NT, S], F32, tag="bigf32", name="xT")
        invb = persist.tile([P, S], F32, tag="invb", name="invb")

        def rms_invb(zT):
            # invb[:, ch*512:...] = 1/sqrt(mean_d z^2 + eps) (rows identical)
            with tc.tile_pool(name="rms_ps", bufs=2, space="PSUM") as rps, \
                 tc.tile_pool(name="rms_sc", bufs=2) as rsc:
                for ch in range(2):
                    ms = None
                    for dt in range(NT):
                        sq = rsc.tile([P, 512], BF16, tag="sqd", name="sqd")
                        eng = nc.vector
                        eng.tensor_mul(
                            sq[:],
                            zT[:, dt, ch * 512 : (ch + 1) * 512],
                            zT[:, dt, ch * 512 : (ch + 1) * 512],
                        )
                        if dt == 0:
                            ms = rps.tile([P, 512], F32, tag="ms", name="ps_ms")
                        nc.tensor.matmul(
                            ms[:], ones128[:], sq[:],
                            start=(dt == 0), stop=(dt == NT - 1),
                        )
                    rms = rsc.tile([P, 512], F32, tag="rms", name="rms")
                    nc.scalar.activation(rms[:], ms[:], AF.Sqrt, bias=eps1[:], scale=1.0 / D)
                    nc.vector.reciprocal_approx_fast(
                        invb[:, ch * 512 : (ch + 1) * 512], rms[:]
                    )

        def modulate(zT, dstT, aa, sh, sc_pool):
            # dstT = 8*(zT*invb*(1+sc) + shift) in fp8 (the *8 is folded into aa/sh)
            for dt in range(NT):
                eng = nc.vector
                tmp = sc_pool.tile([P, S], F32, tag=f"mtmp{dt % 2}", name="mtmp")
                eng.tensor_mul(tmp[:], zT[:, dt, :], invb[:])
                eng.tensor_scalar(
                    dstT[:, dt, :], tmp[:], aa[:, dt : dt + 1], sh[:, dt : dt + 1],
                    op0=ALU.mult, op1=ALU.add,
                )

        # ---- Phase B: load x, transpose to [ch, seq] ----
        with tc.tile_pool(name="xin_pool", bufs=3) as xin_pool, \
             tc.tile_pool(name="psB", bufs=2, space="PSUM") as psB:
            for st in range(NT):
                xin = xin_pool.tile([P, D], F32, tag="xin", name="xin")
                nc.sync.dma_start(out=xin[:], in_=d["x"][st * P : (st + 1) * P, :])
                for g4 in range(2):
                    pt = psB.tile([P, 512], F32, tag="pt", name="ps_tr")
                    for j in range(4):
                        dt = g4 * 4 + j
                        nc.tensor.transpose(
                            pt[:, j * P : (j + 1) * P],
                            xin[:, dt * P : (dt + 1) * P],
                            ident[:],
                        )
                    nc.scalar.activation(
                        xT[:, g4 * 4 : (g4 + 1) * 4, st * P : (st + 1) * P],
                        pt[:], AF.Copy,
                    )

        rms_invb(xT)

        # ---- Phase A: adaLN mods (computed once for both branches) ----
        cT8 = small.tile([P, NT], FP8, name="cT8")
        with tc.tile_pool(name="ada_tmp", bufs=1) as ada_tmp:
            cs = ada_tmp.tile([P, NT], F32, name="cT_silu")
            nc.scalar.activation(cs[:], cT[:], AF.Silu)
            nc.vector.tensor_scalar_mul(cT8[:], cs[:], SH)

        with tc.tile_pool(name="ada_sc", bufs=2) as ada_sc, \
             tc.tile_pool(name="wada_pool", bufs=2) as wada_pool, \
             tc.tile_pool(name="ada_ps", bufs=2, space="PSUM") as ada_ps:
            for n in range(12):
                ps = ada_ps.tile([1, 512], F32, tag="ps", name="ps_ada")
                wt = wada_pool.tile([P, NT, 512], FP8, tag="wada", name="wada_t")
                nc.sync.dma_start(out=wt[:], in_=d["wadaT"][n])
                for kt in range(NT):
                    nc.tensor.matmul(
                        ps[:], cT8[:, kt : kt + 1], wt[:, kt, :],
                        start=(kt == 0), stop=(kt == NT - 1),
                    )
                bch = ada_sc.tile([1, 512], F32, tag="bch", name="bada_ch")
                nc.sync.dma_start(out=bch[:], in_=d["b_ada"][:, n * 512 : (n + 1) * 512])
                mch = ada_sc.tile([1, 512], F32, tag="mch", name="mods_ch")
                nc.vector.scalar_tensor_tensor(
                    mch[:], ps[:], 1.0 / QS, bch[:], op0=ALU.mult, op1=ALU.add
                )
                nc.sync.dma_start(out=mods_dram[:, n * 512 : (n + 1) * 512], in_=mch[:])
        modsT = small.tile([P, 48], F32, name="modsT")
        nc.sync.dma_start(
            out=modsT[:], in_=mods_dram.ap()[0, :].rearrange("(t p) -> p t", p=P)
        )
        if DEBUG:
            nc.sync.dma_start(out=d["dbg_mods"][:], in_=modsT[:])
        # branch-1 (attention) modulation constants, *8 folded for fp8 hT
        a1 = small.tile([P, NT], F32, name="a1")
        nc.vector.tensor_scalar_add(a1[:], modsT[:, 8:16], 1.0)
        nc.vector.tensor_mul(a1[:], a1[:], n1T8[:])
        sh18 = small.tile([P, NT], F32, name="sh18")
        nc.vector.tensor_scalar_mul(sh18[:], modsT[:, 0:8], SH)
        g1 = modsT[:, 16:24]
        g1s = small.tile([P, NT], F32, name="g1s")
        nc.vector.tensor_scalar_mul(g1s[:], g1, 1.0 / (SW * SO))
        g1b = small.tile([P, NT], F32, name="g1b")
        nc.vector.tensor_mul(g1b[:], g1, bprojT[:])
        # branch-2 (MLP)
        a2 = small.tile([P, NT], F32, name="a2")
        nc.vector.tensor_scalar_add(a2[:], modsT[:, 32:40], 1.0)
        nc.vector.tensor_mul(a2[:], a2[:], n2T8[:])
        sh28 = small.tile([P, NT], F32, name="sh28")
        nc.vector.tensor_scalar_mul(sh28[:], modsT[:, 24:32], SH)
        g2 = modsT[:, 40:48]
        g2s = small.tile([P, NT], F32, name="g2s")
        nc.vector.tensor_scalar_mul(g2s[:], g2, 1.0 / (SW * SG))
        g2b3 = small.tile([P, NT], F32, name="g2b3")
        nc.vector.tensor_mul(g2b3[:], g2, b3T[:])

        # ======= attention branch =======
        with ExitStack() as actx:
            ho = actx.enter_context(tc.tile_pool(name="ho", bufs=1))
            ohat = ho.tile([P, NT, S], FP8, tag="ohat", name="ohat")

            with ExitStack() as cctx:
                qk = cctx.enter_context(tc.tile_pool(name="qk", bufs=1))
                qhat = qk.tile([P, NT, S], BF16, tag="qhat", name="qhat")
                khat = qk.tile([P, NT, S], BF16, tag="khat", name="khat")
                v_sb = qk.tile([P, NT, H * 65], FP8, tag="v", name="v_sb")

                hop = cctx.enter_context(tc.tile_pool(name="hop", bufs=1))
                hT = hop.tile([P, NT, S], FP8, tag="hT", name="hT")

                with ExitStack() as qctx:
                    mod_sc = qctx.enter_context(tc.tile_pool(name="mod_sc", bufs=2))
                    modulate(xT, hT, a1, sh18, mod_sc)
                    if DEBUG:
                        nc.sync.dma_start(out=d["dbg_invb"][:], in_=invb[:])
                        nc.sync.dma_start(out=d["dbg_h0"][:], in_=hT[:, 0, :])

                    ropec = qctx.enter_context(tc.tile_pool(name="ropec", bufs=1))
                    qkn = qctx.enter_context(tc.tile_pool(name="qkn", bufs=1))
                    wqk_pool = qctx.enter_context(tc.tile_pool(name="wqk_pool", bufs=3))
                    rope_sc = qctx.enter_context(tc.tile_pool(name="rope_sc", bufs=2))
                    qk_ps = qctx.enter_context(
                        tc.tile_pool(name="qk_ps", bufs=3, space="PSUM")
                    )
                    pe_ps = qctx.enter_context(
                        tc.tile_pool(name="pe_ps", bufs=2, space="PSUM")
                    )
                    ss_ps = qctx.enter_context(
                        tc.tile_pool(name="ss_ps", bufs=2, space="PSUM")
                    )

                    cos2q = load_const("cos2q", [P, S], BF16, pool=ropec)
                    sin2q = load_const("sin2q", [P, S], BF16, pool=ropec)
                    cos2k = load_const("cos2k", [P, S], BF16, pool=ropec)
                    sin2k = load_const("sin2k", [P, S], BF16, pool=ropec)

                    # k tiles first (m 8..15), then q tiles (m 0..7)
                    for m in list(range(NT, 2 * NT)) + list(range(NT)):
                        isq = m < NT
                        mk = m if isq else m - NT
                        wt = wqk_pool.tile([P, NT, P], FP8, tag="wqk", name="wqk_t")
                        nc.sync.dma_start(out=wt[:], in_=d["wqkT"][m])
                        raw = rope_sc.tile([P, S], BF16, tag="raw", name="qk_raw")
                        stage = qkn.tile([2, S], F32, tag="stage", name="stage", bufs=2)
                        for sch in range(2):
                            ps = qk_ps.tile([P, 512], F32, tag="ps", name="ps_qkv")
                            for t in range(4):
                                nc.tensor.matmul(
                                    ps[:], wt[:, 2 * t : 2 * t + 2, :],
                                    hT[:, 2 * t : 2 * t + 2, sch * 512 : (sch + 1) * 512],
                                    start=(t == 0), stop=(t == 3), perf_mode=DR,
                                )
                            if sch == 0:
                                nc.scalar.activation(
                                    raw[:, sch * 512 : (sch + 1) * 512], ps[:],
                                    AF.Identity,
                                    bias=bqkT[:, m : m + 1], scale=1.0 / QS,
                                )
                            else:
                                nc.vector.tensor_scalar(
                                    raw[:, sch * 512 : (sch + 1) * 512], ps[:],
                                    1.0 / QS, bqkT[:, m : m + 1],
                                    op0=ALU.mult, op1=ALU.add,
                                )
                            sqs = rope_sc.tile([P, 512], BF16, tag="sqs", name="sqs")
                            if sch == 0:
                                nc.scalar.activation(
                                    sqs[:], raw[:, sch * 512 : (sch + 1) * 512],
                                    AF.Square,
                                )
                            else:
                                nc.vector.tensor_mul(
                                    sqs[:],
                                    raw[:, sch * 512 : (sch + 1) * 512],
                                    raw[:, sch * 512 : (sch + 1) * 512],
                                )
                            ss = ss_ps.tile([2, 512], F32, tag="ss", name="ps_ss")
                            nc.tensor.matmul(ss[:], bo2[:], sqs[:], start=True, stop=True)
                            nc.scalar.activation(
                                stage[:, sch * 512 : (sch + 1) * 512], ss[:], AF.Sqrt,
                                bias=(eps1[0:2, :] if isq else epsk[0:2, :]),
                                scale=(1.0 / HD if isq else 1.0),
                            )
                        # rope: rotate-half via 32-row block swaps
                        rot = rope_sc.tile([P, S], BF16, tag="rot", name="rot", bufs=2)
                        for blk in range(4):
                            b0 = blk * 32
                            srcb = b0 + (32 if blk % 2 == 0 else -32)
                            nc.gpsimd.dma_start(
                                out=rot[b0 : b0 + 32, :], in_=raw[srcb : srcb + 32, :]
                            )
                        t1 = rope_sc.tile([P, S], BF16, tag="t1", name="rope_t1", bufs=2)
                        t2 = rope_sc.tile([P, S], BF16, tag="t2", name="rope_t2", bufs=2)
                        u = rope_sc.tile([P, S], BF16, tag="u", name="rope_u", bufs=2)
                        nc.vector.tensor_mul(t1[:], raw[:], cos2q[:] if isq else cos2k[:])
                        nc.vector.tensor_mul(t2[:], rot[:], sin2q[:] if isq else sin2k[:])
                        nc.vector.tensor_add(u[:], t1[:], t2[:])
                        # fold inverse-rms (q) / inverse-rms/8 (k) into the tile
                        rstage = qkn.tile([2, S], F32, tag="rstage", name="rstage", bufs=2)
                        nc.vector.reciprocal_approx_fast(rstage[:], stage[:])
                        rbf = qkn.tile([2, S], BF16, tag="rbf", name="rbf", bufs=2)
                        nc.scalar.activation(rbf[:], rstage[:], AF.Copy)
                        dst = qhat if isq else khat
                        for sch in range(2):
                            pe = pe_ps.tile([P, 512], F32, tag="pe", name="ps_erq")
                            nc.tensor.matmul(
                                pe[:], E2[:], rbf[:, sch * 512 : (sch + 1) * 512],
                                start=True, stop=True,
                            )
                            nc.vector.tensor_mul(
                                dst[:, mk, sch * 512 : (sch + 1) * 512],
                                u[:, sch * 512 : (sch + 1) * 512], pe[:],
                            )
                        if DEBUG and mk == 0:
                            nc.sync.dma_start(
                                out=d["dbg_rawq0" if isq else "dbg_rawk0"][:],
                                in_=raw[:],
                            )
                            nc.sync.dma_start(
                                out=d["dbg_qhat0" if isq else "dbg_khat0"][:],
                                in_=dst[:, 0, :],
                            )

                # v (fp8, *16; bias row gives the softmax denominator)
                with tc.tile_pool(name="wv_pool", bufs=2) as wv_pool, \
                     tc.tile_pool(name="v_ps", bufs=3, space="PSUM") as v_ps:
                    for nch in range(4):
                        c0 = nch * 260
                        wt = wv_pool.tile([P, NT, 260], FP8, tag="wv", name="wv_t")
                        nc.sync.dma_start(out=wt[:], in_=d["wvT"][nch])
                        for st in range(NT):
                            ps = v_ps.tile([P, 260], F32, tag="ps", name="ps_v")
                            for t in range(4):
                                nc.tensor.matmul(
                                    ps[:],
                                    hT[:, 2 * t : 2 * t + 2, st * P : (st + 1) * P],
                                    wt[:, 2 * t : 2 * t + 2, :],
                                    start=(t == 0), stop=False, perf_mode=DR,
                                )
                            nc.tensor.matmul(
                                ps[:], ones1[:], bv[:, c0 : c0 + 260],
                                start=False, stop=True, skip_group_check=True,
                            )
                            nc.vector.tensor_scalar_mul(
                                v_sb[:, st, c0 : c0 + 260], ps[:], SV / QS
                            )
                    if DEBUG:
                        nc.sync.dma_start(out=d["dbg_v0"][:], in_=v_sb[:, 0, :])

                # ---- attention ----
                with tc.tile_pool(name="psA", bufs=1, space="PSUM") as psA, \
                     tc.tile_pool(name="psB2", bufs=1, space="PSUM") as psB2, \
                     tc.tile_pool(name="av_ps", bufs=1, space="PSUM") as av_ps, \
                     tc.tile_pool(name="pb_ps", bufs=1, space="PSUM") as pb_ps, \
                     tc.tile_pool(name="pT_pool", bufs=2) as pT_pool, \
                     tc.tile_pool(name="att_sc", bufs=2) as att_sc:

                    def qk_exp(qch, h):
                        mk, hh = h // 2, h % 2
                        rb = 64 * hh
                        pT = pT_pool.tile([P, NT, 512], FP8, tag="pT", name="pT")
                        grpA = psA.tile([P, 3, 512], F32, tag="gA", name="ps_gA")
                        for j in range(3):
                            nc.tensor.matmul(
                                grpA[:, j, :],
                                khat[rb : rb + 64, mk, j * P : (j + 1) * P],
                                qhat[rb : rb + 64, mk, qch * 512 : (qch + 1) * 512],
                                start=True, stop=True,
                            )
                        grpB = psB2.tile([P, 3, 512], F32, tag="gB", name="ps_gB")
                        for j in range(3):
                            nc.tensor.matmul(
                                grpB[:, j, :],
                                khat[rb : rb + 64, mk, (j + 3) * P : (j + 4) * P],
                                qhat[rb : rb + 64, mk, qch * 512 : (qch + 1) * 512],
                                start=True, stop=True,
                            )
                        nc.scalar.activation(
                            pT[:, 0:3, :], grpA[:], AF.Exp, bias=0.0, scale=1.0
                        )
                        grpC = psA.tile([P, 3, 512], F32, tag="gA", name="ps_gC")
                        for j in range(2):
                            nc.tensor.matmul(
                                grpC[:, j, :],
                                khat[rb : rb + 64, mk, (j + 6) * P : (j + 7) * P],
                                qhat[rb : rb + 64, mk, qch * 512 : (qch + 1) * 512],
                                start=True, stop=True,
                            )
                        nc.scalar.activation(
                            pT[:, 3:6, :], grpB[:], AF.Exp, bias=0.0, scale=1.0
                        )
                        nc.scalar.activation(
                            pT[:, 6:8, :], grpC[:, 0:2, :], AF.Exp, bias=0.0,
                            scale=1.0,
                        )
                        return pT

                    def av_div(qch, h, pT):
                        mk, hh = h // 2, h % 2
                        rb = 64 * hh
                        av = av_ps.tile([65, 512], F32, tag="av", name="ps_av")
                        for t in range(4):
                            nc.tensor.matmul(
                                av[:], v_sb[:, 2 * t : 2 * t + 2, h * 65 : h * 65 + 65],
                                pT[:, 2 * t : 2 * t + 2, :],
                                start=(t == 0), stop=(t == 3), perf_mode=DR,
                            )
                        o65b = att_sc.tile([65, 512], BF16, tag="o65b", name="o65b")
                        nc.vector.tensor_copy(o65b[:], av[:])
                        pb = pb_ps.tile([64, 512], F32, tag="pb", name="ps_pb")
                        nc.tensor.matmul(pb[:], e65[:], o65b[:], start=True, stop=True)
                        rb64 = att_sc.tile([64, 512], F32, tag="rb64", name="rb64")
                        nc.vector.reciprocal_approx_fast(rb64[:], pb[:])
                        ob = att_sc.tile([64, 512], FP8, tag="ob", name="ob")
                        nc.vector.scalar_tensor_tensor(
                            ob[:], o65b[0:64, :], SO, rb64[:], op0=ALU.mult, op1=ALU.mult
                        )
                        nc.sync.dma_start(
                            out=ohat[rb : rb + 64, mk, qch * 512 : (qch + 1) * 512],
                            in_=ob[:],
                        )

                    prev = None
                    for qch in range(2):
                        for h in range(H):
                            if prev is not None:
                                av_div(*prev)
                            pT = qk_exp(qch, h)
                            prev = (qch, h, pT)
                    av_div(*prev)

            # ---- proj + residual 1 (in place on xT) ----
            with tc.tile_pool(name="wproj_pool", bufs=3) as wproj_pool, \
                 tc.tile_pool(name="pj_ps", bufs=3, space="PSUM") as pj_ps:
                for dt in range(NT):
                    wt = wproj_pool.tile([P, NT, P], FP8, tag="wproj", name="wproj_t")
                    nc.sync.dma_start(out=wt[:], in_=d["wprojT"][dt])
                    for qch in range(2):
                        ps = pj_ps.tile([P, 512], F32, tag="ps", name="ps_proj")
                        for t in range(4):
                            nc.tensor.matmul(
                                ps[:], wt[:, 2 * t : 2 * t + 2, :],
                                ohat[:, 2 * t : 2 * t + 2, qch * 512 : (qch + 1) * 512],
                                start=(t == 0), stop=(t == 3), perf_mode=DR,
                            )
                        nc.vector.affine_then_add(
                            xT[:, dt, qch * 512 : (qch + 1) * 512],
                            ps[:], xT[:, dt, qch * 512 : (qch + 1) * 512],
                            scale=g1s[:, dt : dt + 1], bias=g1b[:, dt : dt + 1],
                        )

        # ======= MLP branch =======
        with ExitStack() as mctx:
            mlp = mctx.enter_context(tc.tile_pool(name="mlp", bufs=1))

            rms_invb(xT)

            h2T = mlp.tile([P, NT, S], FP8, tag="h2T", name="h2T")
            with tc.tile_pool(name="mod_sc2", bufs=2) as mod_sc2:
                modulate(xT, h2T, a2, sh28, mod_sc2)

            gg = mlp.tile([P, NKT12, S], FP8, tag="gg", name="gg")
            with tc.tile_pool(name="w12_pool", bufs=3) as w12_pool, \
                 tc.tile_pool(name="mlp_sc", bufs=2) as mlp_sc, \
                 tc.tile_pool(name="mlp_ps", bufs=2, space="PSUM") as mlp_ps:
                for j in range(NKT12):
                    outs = []
                    for part in range(2):
                        wt = w12_pool.tile([P, NT, P], FP8, tag="w12", name="w12_t")
                        nc.sync.dma_start(out=wt[:], in_=d["w12T"][part * NKT12 + j])
                        ps2 = mlp_ps.tile([P, 2, 512], F32, tag="ps2", name="ps_mlp")
                        for sch in range(2):
                            for t in range(4):
                                nc.tensor.matmul(
                                    ps2[:, sch, :], wt[:, 2 * t : 2 * t + 2, :],
                                    h2T[:, 2 * t : 2 * t + 2, sch * 512 : (sch + 1) * 512],
                                    start=(t == 0), stop=(t == 3), perf_mode=DR,
                                )
                        o = mlp_sc.tile([P, S], BF16, tag=f"mlp{part}", name=f"mlp{part}")
                        if part == 0:
                            nc.scalar.activation(
                                o[:], ps2[:], AF.Silu,
                                bias=b12T[:, j : j + 1], scale=1.0 / QS,
                            )
                        else:
                            nc.vector.tensor_scalar(
                                o[:], ps2[:], SG / QS, b12T[:, NKT12 + j : NKT12 + j + 1],
                                op0=ALU.mult, op1=ALU.add,
                            )
                        outs.append(o)
                    nc.vector.tensor_mul(gg[:, j, :], outs[0][:], outs[1][:])

            # w3 + residual 2 (in place on xT)
            with tc.tile_pool(name="w3_pool", bufs=2) as w3_pool, \
                 tc.tile_pool(name="w3_ps", bufs=3, space="PSUM") as w3_ps:
                for dt in range(NT):
                    wt = w3_pool.tile([P, NKT12, P], FP8, tag="w3", name="w3_t")
                    nc.sync.dma_start(out=wt[:], in_=d["w3T"][dt])
                    for qch in range(2):
                        ps = w3_ps.tile([P, 512], F32, tag="ps", name="ps_w3")
                        for t in range(11):
                            nc.tensor.matmul(
                                ps[:], wt[:, 2 * t : 2 * t + 2, :],
                                gg[:, 2 * t : 2 * t + 2, qch * 512 : (qch + 1) * 512],
                                start=(t == 0), stop=(t == 10), perf_mode=DR,
                            )
                        nc.vector.affine_then_add(
                            xT[:, dt, qch * 512 : (qch + 1) * 512],
                            ps[:], xT[:, dt, qch * 512 : (qch + 1) * 512],
                            scale=g2s[:, dt : dt + 1], bias=g2b3[:, dt : dt + 1],
                        )

            # ---- output transpose ----
            with tc.tile_pool(name="yout", bufs=3) as ypool, \
                 tc.tile_pool(name="psH", bufs=2, space="PSUM") as psH:
                for st in range(NT):
                    y = ypool.tile([P, D], F32, tag="y", name="y")
                    for g4 in range(2):
                        pt = psH.tile([P, 512], F32, tag="pt", name="ps_tr2")
                        for j in range(4):
                            dt = g4 * 4 + j
                            nc.tensor.transpose(
                                pt[:, j * P : (j + 1) * P],
                                xT[:, dt, st * P : (st + 1) * P],
                                ident[:],
                            )
                        nc.scalar.activation(
                            y[:, g4 * 512 : (g4 + 1) * 512], pt[:], AF.Copy
                        )
                    nc.sync.dma_start(out=d["out"][st * P : (st + 1) * P, :], in_=y[:])


def kernel(**inputs):
    inputs = {k: np.asarray(v) for k, v in inputs.items()}
    if "nc" not in _CACHE:
        _CACHE["nc"] = build_bass()
    nc = _CACHE["nc"]

    base = _prep_weights(inputs)

    in_maps = []
    for core in range(B):
        m = dict(base)
        m["x"] = np.ascontiguousarray(inputs["x"][core]).astype(np.float32)
        m["cT"] = _to_pmaj(inputs["c"][core]).astype(np.float32)
        in_maps.append(m)

    res = run_bass_kernel_spmd(
        nc, in_maps, core_ids=list(range(B)), **_CACHE.get("run_kwargs", {})
    )
    _CACHE["last_results"] = res
    return np.stack([res.results[i]["out"] for i in range(B)], axis=0)


if __name__ == "__main__":
    build_bass()
    print("built ok")
